# revision 1
# baseline (speedup 1.0000x reference)
"""Trainium2 Bass kernel for fused attention (QKV proj + RoPE + SDPA + o_proj).

Sharding: Megatron-style tensor parallel over heads (4 heads/core x 8 cores)
for QKV+SDPA, then an AllToAll switches to token parallelism for o_proj, so
each core emits a disjoint slice of the final output (host just concatenates).

All device matmuls run as float32r (full-rate fp32 on the PE array, ~1e-4 rel).
Activations stay in transposed [e, t] layouts end-to-end so no on-device
activation transposes are needed.
"""
import sys

import numpy as np

try:
    import concourse.bass as bass
except ImportError:  # fresh grading env: make the toolchain importable
    for p in (
        "/root/.axon_site",
        "/root/.axon_site/_ro/trn_rl_repo",
        "/root/.axon_site/_ro/pypackages",
        "/opt/trn_rl_repo",
        "/opt/pypackages",
    ):
        if p not in sys.path:
            sys.path.append(p)
    import concourse.bass as bass

import concourse.bacc as bacc
import concourse.mybir as mybir
import concourse.tile as tile
from concourse.bass_utils import run_bass_kernel_spmd

F32 = mybir.dt.float32
F32R = mybir.dt.float32r
MULT = mybir.AluOpType.mult
ADD = mybir.AluOpType.add

# problem dims (hardcoded for nn_Attention_42846593744909)
B, S, D = 4, 1024, 2048
H, HD = 32, 64
N_CORES = 8
H_LOC = H // N_CORES  # heads per core


def build_attention(b=B, s=S, d=D, h_loc=H_LOC, hd=HD, n_cores=N_CORES):
    """Build the per-core SPMD Bass program. Returns finalized nc."""
    P = 128
    T = b * s                 # total tokens
    TS = T // n_cores         # output token slice per core
    DCH = d // P              # contraction chunks for D
    QBLK = h_loc * hd         # 256: q (or k, or v) width per core
    NQKQ = QBLK // P          # q e-chunks (2)
    NQK = 2 * NQKQ            # q+k e-chunks (4)
    EVA = h_loc * (hd + 1)    # v + ones columns (260)
    TCH = min(256, s)         # qkv token chunk
    NTC = s // TCH
    QT = min(512, s, TS)      # query-tile width in SDPA
    NQT = s // QT
    KTC = s // P              # key chunks of 128
    ECH = n_cores * QBLK // P  # o_proj contraction chunks (16)
    NH = 2                    # number of A2A rounds (token halves)
    SH = T // (NH * n_cores)  # shard tokens per core per half
    TS_H = TS // NH
    ODC = min(256, d)         # o_proj dout chunk
    TSUB = TS // P            # o_proj token subchunks
    NP = min(NH, TSUB)        # o_proj passes
    assert QT <= TS and TS % QT == 0 and QT % SH == 0 and SH == TS_H
    assert TSUB % NP == 0

    nc = bacc.Bacc()
    hidden_t = nc.dram_tensor("hidden_t", [d, T], F32R, kind="ExternalInput")
    w_qk_t = nc.dram_tensor("w_qk_t", [d, 2 * QBLK], F32R, kind="ExternalInput")
    w_v_t = nc.dram_tensor("w_v_t", [d, QBLK], F32R, kind="ExternalInput")
    w_o_t = nc.dram_tensor("w_o_t", [n_cores * QBLK, d], F32R, kind="ExternalInput")
    cos2 = nc.dram_tensor("cos2", [P, s], F32, kind="ExternalInput")
    sinrot2 = nc.dram_tensor("sinrot2", [P, s], F32, kind="ExternalInput")
    out_sl = nc.dram_tensor("out_sl", [TS, d], F32, kind="ExternalOutput")

    hid_v = hidden_t[:].rearrange("(c p) t -> p c t", p=P)
    wqk_v = w_qk_t[:].rearrange("(c p) e -> p c e", p=P)
    wv_v = w_v_t[:].rearrange("(c p) e -> p c e", p=P)
    wo_v = w_o_t[:].rearrange("(c p) e -> p c e", p=P)

    with tile.TileContext(nc) as tc:
        with tc.tile_pool(name="dramp", bufs=1, space="DRAM") as dramp:
            cc_in_h = [dramp.tile([n_cores, QBLK, SH], F32, name=f"cc_in_{h}")
                       for h in range(NH)]
            cc_out_h = [dramp.tile([n_cores, QBLK, SH], F32, name=f"cc_out_{h}")
                        for h in range(NH)]

            with (
                tc.tile_pool(name="tabs", bufs=1) as tabs,
                tc.tile_pool(name="hidp", bufs=2) as hidp,
                tc.tile_pool(name="qkp", bufs=2) as qkp,
                tc.tile_pool(name="vp", bufs=2) as vp,
                tc.tile_pool(name="ropep", bufs=2) as ropep,
                tc.tile_pool(name="expp", bufs=3) as expp,
                tc.tile_pool(name="normp", bufs=2) as normp,
                tc.tile_pool(name="drowp", bufs=4, space="DRAM") as drowp,
                tc.tile_pool(name="psA", bufs=2, space="PSUM") as psA,
                tc.tile_pool(name="psS", bufs=3, space="PSUM") as psS,
                tc.tile_pool(name="psO", bufs=3, space="PSUM") as psO,
            ):
                cos_sb = tabs.tile([P, s], F32)
                sin_sb = tabs.tile([P, s], F32)
                nc.sync.dma_start(cos_sb[:], cos2[:])
                nc.sync.dma_start(sin_sb[:], sinrot2[:])

                with tc.tile_pool(name="wqkp", bufs=1) as wqkp:
                    wqk_sb = wqkp.tile([P, DCH, 2 * QBLK], F32R)
                    wv_sb = wqkp.tile([P, DCH, QBLK], F32R)
                    wstep = max(1, DCH // 4)
                    for dd4 in range(0, DCH, wstep):
                        nc.sync.dma_start(wqk_sb[:, dd4:dd4 + wstep],
                                          wqk_v[:, dd4:dd4 + wstep])
                        nc.sync.dma_start(wv_sb[:, dd4:dd4 + wstep],
                                          wv_v[:, dd4:dd4 + wstep])

                    for bi in range(b):
                        # ---- QKV projection + RoPE for batch bi ----
                        qk_t = qkp.tile([P, NQK, s], F32R, tag="qk")
                        v_t = vp.tile([P, KTC, EVA], F32R, tag="v")
                        for h in range(h_loc):
                            nc.scalar.activation(
                                v_t[:, :, h * (hd + 1) + hd:h * (hd + 1) + hd + 1],
                                wv_sb[:, 0:KTC, 0:1],
                                mybir.ActivationFunctionType.Identity,
                                bias=1.0, scale=0.0,
                            )

                        for tci in range(NTC):
                            t0 = bi * s + tci * TCH
                            s0 = tci * TCH
                            hid_sb = hidp.tile([P, DCH, TCH], F32R, tag="hid")
                            nc.sync.dma_start(hid_sb[:], hid_v[:, :, t0:t0 + TCH])

                            for ec in range(NQK):
                                ps = psA.tile([P, max(QBLK, TCH)], F32, tag="psqk",
                                              name="psqk")[:, 0:TCH]
                                for dd in range(DCH):
                                    nc.tensor.matmul(
                                        ps[:], lhsT=wqk_sb[:, dd, ec * P:(ec + 1) * P],
                                        rhs=hid_sb[:, dd, :],
                                        start=(dd == 0), stop=(dd == DCH - 1),
                                    )
                                # RoPE: out = ps*cos + swap32(ps)*sinrot
                                raw = ropep.tile([P, TCH], F32, tag="raw")
                                nc.any.tensor_copy(raw[:], ps[:])
                                cp = ropep.tile([P, TCH], F32, tag="cp")
                                nc.vector.tensor_tensor(cp[:], ps[:], cos_sb[:, s0:s0 + TCH], MULT)
                                sw = ropep.tile([P, TCH], F32, tag="sw")
                                nc.sync.dma_start(sw[0:32, :], raw[32:64, :])
                                nc.sync.dma_start(sw[32:64, :], raw[0:32, :])
                                nc.sync.dma_start(sw[64:96, :], raw[96:128, :])
                                nc.sync.dma_start(sw[96:128, :], raw[64:96, :])
                                nc.vector.tensor_tensor(sw[:], sw[:], sin_sb[:, s0:s0 + TCH], MULT)
                                nc.vector.tensor_tensor(qk_t[:, ec, s0:s0 + TCH], cp[:], sw[:], ADD)

                            for tsub in range(TCH // P):
                                kc = tci * (TCH // P) + tsub
                                psv = psA.tile([P, max(QBLK, TCH)], F32, tag="psqk",
                                               name="psv")[:, 0:QBLK]
                                for dd in range(DCH):
                                    nc.tensor.matmul(
                                        psv[:], lhsT=hid_sb[:, dd, tsub * P:(tsub + 1) * P],
                                        rhs=wv_sb[:, dd, :],
                                        start=(dd == 0), stop=(dd == DCH - 1),
                                    )
                                for h in range(h_loc):
                                    nc.any.tensor_copy(
                                        v_t[:, kc, h * (hd + 1):h * (hd + 1) + hd],
                                        psv[:, h * hd:(h + 1) * hd],
                                    )

                        # ---- SDPA for batch bi ----
                        for pp in range(h_loc // 2):
                            for qt in range(NQT):
                                q0 = qt * QT
                                ps_o0 = psO.tile([P, QT], F32, tag="pso")
                                ps_o1 = psO.tile([P, QT], F32, tag="pso")
                                for kt in range(KTC):
                                    ps_s0 = psS.tile([P, QT], F32, tag="pss")
                                    ps_s1 = psS.tile([P, QT], F32, tag="pss")
                                    nc.tensor.matmul(
                                        ps_s0[:],
                                        lhsT=qk_t[0:64, NQKQ + pp, kt * P:(kt + 1) * P],
                                        rhs=qk_t[0:64, pp, q0:q0 + QT],
                                        start=True, stop=True,
                                    )
                                    nc.tensor.matmul(
                                        ps_s1[:],
                                        lhsT=qk_t[64:128, NQKQ + pp, kt * P:(kt + 1) * P],
                                        rhs=qk_t[64:128, pp, q0:q0 + QT],
                                        start=True, stop=True, tile_position=(64, 0),
                                    )
                                    e0 = expp.tile([P, QT], F32R, tag="exp")
                                    e1 = expp.tile([P, QT], F32R, tag="exp")
                                    nc.scalar.activation(e0[:], ps_s0[:], mybir.ActivationFunctionType.Exp)
                                    nc.scalar.activation(e1[:], ps_s1[:], mybir.ActivationFunctionType.Exp)
                                    h0 = 2 * pp
                                    h1 = 2 * pp + 1
                                    nc.tensor.matmul(
                                        ps_o0[0:hd + 1, :],
                                        lhsT=v_t[:, kt, h0 * (hd + 1):(h0 + 1) * (hd + 1)],
                                        rhs=e0[:],
                                        start=(kt == 0), stop=(kt == KTC - 1),
                                    )
                                    nc.tensor.matmul(
                                        ps_o1[0:hd + 1, :],
                                        lhsT=v_t[:, kt, h1 * (hd + 1):(h1 + 1) * (hd + 1)],
                                        rhs=e1[:],
                                        start=(kt == 0), stop=(kt == KTC - 1),
                                    )
                                gq0 = bi * s + qt * QT
                                # pair-batched softmax denominators
                                dcp0 = normp.tile([hd + 1, QT], F32, tag="dcp")
                                dcp1 = normp.tile([hd + 1, QT], F32, tag="dcp")
                                nc.scalar.copy(dcp0[hd:hd + 1, :], ps_o0[hd:hd + 1, :])
                                nc.scalar.copy(dcp1[hd:hd + 1, :], ps_o1[hd:hd + 1, :])
                                dg = normp.tile([2, QT], F32, tag="dg")
                                nc.sync.dma_start(dg[0:1, :], dcp0[hd:hd + 1, :])
                                nc.sync.dma_start(dg[1:2, :], dcp1[hd:hd + 1, :])
                                dgr = normp.tile([2, QT], F32, tag="dgr")
                                nc.vector.reciprocal(dgr[:], dg[:])
                                rd = drowp.tile([2, QT], F32, tag="drow")
                                nc.sync.dma_start(rd[:], dgr[:])
                                for idx, (hh, ps_o) in enumerate(
                                        ((2 * pp, ps_o0), (2 * pp + 1, ps_o1))):
                                    rep = normp.tile([hd, QT], F32, tag="rep")
                                    nc.sync.dma_start(rep[:], rd[idx:idx + 1, :].to_broadcast((hd, QT)))
                                    ao = normp.tile([hd, QT], F32, tag="ao")
                                    nc.vector.tensor_tensor(ao[:], ps_o[0:hd, :], rep[:], MULT)
                                    for w in range(QT // SH):
                                        tok0 = gq0 + w * SH
                                        half = tok0 // (T // NH)
                                        o = tok0 % (T // NH)
                                        nc.sync.dma_start(
                                            cc_in_h[half][o // SH, hh * hd:(hh + 1) * hd, :],
                                            ao[:, w * SH:(w + 1) * SH],
                                        )

                # wqk/wv SBUF released here -> o_proj pools can alias it
                for h in range(NH):
                    nc.gpsimd.collective_compute(
                        "AllToAll",
                        mybir.AluOpType.bypass,
                        replica_groups=[list(range(n_cores))],
                        ins=[cc_in_h[h].opt()],
                        outs=[cc_out_h[h].opt()],
                    )

                # ---- o_proj on this core's token slice, NP passes ----
                with (
                    tc.tile_pool(name="aslp", bufs=1) as aslp,
                    tc.tile_pool(name="wop", bufs=2) as wop,
                    tc.tile_pool(name="osbp", bufs=3) as osbp,
                ):
                    asl = aslp.tile([P, ECH, TS], F32R)
                    for h in range(NH):
                        cc_v = cc_out_h[h][:].rearrange("j (ci p) t -> p (j ci) t", p=P)
                        nc.sync.dma_start(asl[:, :, h * TS_H:(h + 1) * TS_H], cc_v.bitcast(F32R))
                    for hp in range(NP):
                        for dc in range(d // ODC):
                            wo_sb = wop.tile([P, ECH, ODC], F32R, tag="wo")
                            nc.sync.dma_start(wo_sb[:], wo_v[:, :, dc * ODC:(dc + 1) * ODC])
                            for tsub in range(hp * (TSUB // NP), (hp + 1) * (TSUB // NP)):
                                pso = psA.tile([P, max(QBLK, TCH)], F32, tag="psqk",
                                               name="pso")[:, 0:ODC]
                                for e in range(ECH):
                                    nc.tensor.matmul(
                                        pso[:], lhsT=asl[:, e, tsub * P:(tsub + 1) * P],
                                        rhs=wo_sb[:, e, :],
                                        start=(e == 0), stop=(e == ECH - 1),
                                    )
                                ob = osbp.tile([P, ODC], F32, tag="ob")
                                nc.scalar.copy(ob[:], pso[:])
                                nc.sync.dma_start(
                                    out_sl[tsub * P:(tsub + 1) * P, dc * ODC:(dc + 1) * ODC], ob[:]
                                )
    nc.finalize()
    return nc



def prep_inputs(cos, sin, hidden_states, w_qkv, w_o,
                b=B, s=S, d=D, h_loc=H_LOC, hd=HD, n_cores=N_CORES):
    """Host-side sharding/layout: returns per-core input maps."""
    cos = np.asarray(cos, dtype=np.float32)
    sin = np.asarray(sin, dtype=np.float32)
    hidden_states = np.asarray(hidden_states, dtype=np.float32)
    w_qkv = np.asarray(w_qkv, dtype=np.float32)
    w_o = np.asarray(w_o, dtype=np.float32)

    T = b * s
    QBLK = h_loc * hd
    HHD = n_cores * QBLK  # total H*HD

    hidden_t = np.ascontiguousarray(hidden_states.reshape(T, d).T)
    w_o_t = np.ascontiguousarray(w_o.T)

    cos_t = cos.T  # [hd, s]
    sin_t = sin.T
    cos2 = np.ascontiguousarray(np.tile(cos_t, (128 // hd, 1)))
    srt = sin_t.copy()
    srt[0:hd // 2] = -sin_t[0:hd // 2]
    sinrot2 = np.ascontiguousarray(np.tile(srt, (128 // hd, 1)))

    maps = []
    for c in range(n_cores):
        wq = w_qkv[c * QBLK:(c + 1) * QBLK] * 0.125
        wk = w_qkv[HHD + c * QBLK:HHD + (c + 1) * QBLK]
        wv = w_qkv[2 * HHD + c * QBLK:2 * HHD + (c + 1) * QBLK]
        w_qk_t = np.ascontiguousarray(np.concatenate([wq, wk], axis=0).T)
        w_v_t = np.ascontiguousarray(wv.T)
        maps.append({
            "hidden_t": hidden_t,
            "w_qk_t": w_qk_t,
            "w_v_t": w_v_t,
            "w_o_t": w_o_t,
            "cos2": cos2,
            "sinrot2": sinrot2,
        })
    return maps


_NC_CACHE = {}


def run(inputs, trace=False, dims=None):
    """Run the distributed kernel. Returns (full_output, BassKernelResults)."""
    dims = dims or dict(b=B, s=S, d=D, h_loc=H_LOC, hd=HD, n_cores=N_CORES)
    key = tuple(sorted(dims.items()))
    if key not in _NC_CACHE:
        _NC_CACHE[key] = build_attention(**dims)
    nc = _NC_CACHE[key]
    maps = prep_inputs(inputs["cos"], inputs["sin"], inputs["hidden_states"],
                       inputs["w_qkv"], inputs["w_o"], **dims)
    res = run_bass_kernel_spmd(nc, maps, list(range(dims["n_cores"])), trace=trace)
    n_cores = dims["n_cores"]
    T = dims["b"] * dims["s"]
    TS_H = T // (2 * n_cores)
    out = np.empty((T, dims["d"]), dtype=np.float32)
    for c in range(n_cores):
        sl = res.results[c]["out_sl"]
        for h in range(2):
            out[h * (T // 2) + c * TS_H: h * (T // 2) + (c + 1) * TS_H] =                 sl[h * TS_H:(h + 1) * TS_H]
    out = out.reshape(dims["b"], dims["s"], dims["d"])
    return out, res


def kernel(**inputs) -> np.ndarray:
    out, _ = run(inputs)
    return out



# revision 10
# speedup vs baseline: 1.3000x; 1.3000x over previous
"""Trainium2 Bass kernel for fused attention (QKV proj + RoPE + SDPA + o_proj).

Sharding: Megatron-style tensor parallel over heads (4 heads/core x 8 cores)
for QKV+SDPA, then per-batch AllToAll rounds switch to token parallelism for
o_proj, so each core emits a disjoint [d, tokens] slice of the final output
(host transposes + concatenates).

The whole pipeline runs in bf16 on the PE array (fp32 PSUM accumulation):
bf16 matmuls stream at 1 row/cycle vs ~2 for fp32r, and all DMA traffic is
halved. QKV for batch b+1 is software-pipelined into SDPA(b)'s exp-bound
stretches so the PE stays busy while ACT computes exp.
"""
import sys

import numpy as np

try:
    import concourse.bass as bass
except ImportError:  # fresh grading env: make the toolchain importable
    for p in (
        "/root/.axon_site",
        "/root/.axon_site/_ro/trn_rl_repo",
        "/root/.axon_site/_ro/pypackages",
        "/opt/trn_rl_repo",
        "/opt/pypackages",
    ):
        if p not in sys.path:
            sys.path.append(p)
    import concourse.bass as bass

import ml_dtypes

import concourse.bacc as bacc
import concourse.mybir as mybir
import concourse.tile as tile
from concourse.bass_utils import run_bass_kernel_spmd

F32 = mybir.dt.float32
BF16 = mybir.dt.bfloat16
MULT = mybir.AluOpType.mult
ADD = mybir.AluOpType.add
EXP = mybir.ActivationFunctionType.Exp

# problem dims (hardcoded for nn_Attention_42846593744909)
B, S, D = 4, 1024, 2048
H, HD = 32, 64
N_CORES = 8
H_LOC = H // N_CORES  # heads per core


def build_attention(b=B, s=S, d=D, h_loc=H_LOC, hd=HD, n_cores=N_CORES):
    """Build the per-core SPMD Bass program. Returns finalized nc."""
    P = 128
    T = b * s                 # total tokens
    TS = T // n_cores         # output token slice per core
    DCH = d // P              # contraction chunks for D (16)
    QBLK = h_loc * hd         # 256: q (or k, or v) width per core
    EVA = h_loc * (hd + 1)    # v + ones columns (260)
    TCH = 512                 # qkv token chunk
    NTC = s // TCH            # 2 chunks per batch
    QT = 512                  # query-tile width in SDPA
    NQT = s // QT             # 2
    KTC = s // P              # key chunks of 128 (8)
    ECH = n_cores * QBLK // P  # o_proj contraction chunks (16)
    SH = s // n_cores         # shard tokens per core per round (128)
    NDC = d // P              # o_proj output-dim chunks (16)

    nc = bacc.Bacc()
    hidden_t = nc.dram_tensor("hidden_t", [d, T], BF16, kind="ExternalInput")
    w_qk_t = nc.dram_tensor("w_qk_t", [d, 2 * QBLK], BF16, kind="ExternalInput")
    w_v_t = nc.dram_tensor("w_v_t", [d, QBLK], BF16, kind="ExternalInput")
    w_o_t = nc.dram_tensor("w_o_t", [n_cores * QBLK, d], BF16, kind="ExternalInput")
    cos2 = nc.dram_tensor("cos2", [P, s], BF16, kind="ExternalInput")
    sinrot2 = nc.dram_tensor("sinrot2", [P, s], BF16, kind="ExternalInput")
    # output in [d, tokens] layout; host transposes
    out_sl = nc.dram_tensor("out_sl", [d, TS], F32, kind="ExternalOutput")

    hid_v = hidden_t[:].rearrange("(c p) t -> p c t", p=P)
    wqk_v = w_qk_t[:].rearrange("(c p) e -> p c e", p=P)
    wv_v = w_v_t[:].rearrange("(c p) e -> p c e", p=P)
    wo_v = w_o_t[:].rearrange("(c p) e -> p c e", p=P)

    with tile.TileContext(nc) as tc:
        with tc.tile_pool(name="dramp", bufs=1, space="DRAM") as dramp:
            cc_in_h = [dramp.tile([n_cores, QBLK, SH], BF16, name=f"cc_in_{h}")
                       for h in range(b)]
            cc_out_h = [dramp.tile([n_cores, QBLK, SH], BF16, name=f"cc_out_{h}")
                        for h in range(b)]

            with (
                tc.tile_pool(name="tabs", bufs=1) as tabs,
                tc.tile_pool(name="wqkp", bufs=1) as wqkp,
                tc.tile_pool(name="hidp", bufs=3) as hidp,
                tc.tile_pool(name="qkp", bufs=2) as qkp,
                tc.tile_pool(name="vp", bufs=2) as vp,
                tc.tile_pool(name="ropep", bufs=2) as ropep,
                tc.tile_pool(name="expp", bufs=3) as expp,
                tc.tile_pool(name="normp", bufs=4) as normp,
                tc.tile_pool(name="stagep", bufs=2) as stagep,
                tc.tile_pool(name="drowp", bufs=4, space="DRAM") as drowp,
                tc.tile_pool(name="psMM", bufs=2, space="PSUM") as psMM,
                tc.tile_pool(name="psS", bufs=1, space="PSUM") as psS,
                tc.tile_pool(name="psO", bufs=2, space="PSUM") as psO,
                tc.tile_pool(name="aslp", bufs=1) as aslp,
                tc.tile_pool(name="wop", bufs=2) as wop,
                tc.tile_pool(name="osbp", bufs=3) as osbp,
            ):
                cos_sb = tabs.tile([P, s], BF16)
                sin_sb = tabs.tile([P, s], BF16)
                nc.sync.dma_start(cos_sb[:], cos2[:])
                nc.sync.dma_start(sin_sb[:], sinrot2[:])

                wqk_sb = wqkp.tile([P, DCH, 2 * QBLK], BF16)
                wv_sb = wqkp.tile([P, DCH, QBLK], BF16)
                wstep = DCH // 4
                for dd4 in range(0, DCH, wstep):
                    nc.sync.dma_start(wqk_sb[:, dd4:dd4 + wstep],
                                      wqk_v[:, dd4:dd4 + wstep])
                    nc.sync.dma_start(wv_sb[:, dd4:dd4 + wstep],
                                      wv_v[:, dd4:dd4 + wstep])

                def start_qkv(bi):
                    """Allocate batch-bi tiles, start hid DMAs; return
                    (qk_t, v_t, generator-of-remaining-work)."""
                    qk_t = qkp.tile([P, 4, s], BF16, tag="qk", name="qk_t")
                    v_t = vp.tile([P, KTC, EVA], BF16, tag="v", name="v_t")
                    hid_halves = []
                    for tci in range(NTC):
                        t0 = bi * s + tci * TCH
                        hid_sb = hidp.tile([P, DCH, TCH], BF16, tag="hid",
                                           name="hid_sb")
                        nc.sync.dma_start(hid_sb[:], hid_v[:, :, t0:t0 + TCH])
                        hid_halves.append(hid_sb)
                    # ones columns for the softmax-denominator trick
                    for hh in range(h_loc):
                        nc.scalar.activation(
                            v_t[:, :, hh * (hd + 1) + hd:hh * (hd + 1) + hd + 1],
                            wv_sb[:, 0:KTC, 0:1],
                            mybir.ActivationFunctionType.Identity,
                            bias=1.0, scale=0.0,
                        )

                    def work():
                        # V projection: [tokens, e] layout, 128-token groups
                        for tsub in range(KTC):
                            hid_sb = hid_halves[tsub // (TCH // P)]
                            toff = (tsub % (TCH // P)) * P
                            psv = psMM.tile([P, TCH], F32, tag="mm",
                                            name="psv")[:, 0:QBLK]
                            for dd in range(DCH):
                                nc.tensor.matmul(
                                    psv[:], lhsT=hid_sb[:, dd, toff:toff + P],
                                    rhs=wv_sb[:, dd, :],
                                    start=(dd == 0), stop=(dd == DCH - 1),
                                )
                                if dd == DCH // 2:
                                    yield
                            nc.scalar.copy(
                                v_t[:, tsub]
                                    .rearrange("p (h e) -> p h e", e=hd + 1)
                                    [:, :, 0:hd],
                                psv[:].rearrange("p (h e) -> p h e", e=hd),
                            )
                            yield
                        # QK projection + RoPE, [e, tokens] layout
                        for ec in range(4):
                            raw = ropep.tile([P, s], BF16, tag="raw", name="raw")
                            for tci in range(NTC):
                                s0 = tci * TCH
                                ps = psMM.tile([P, TCH], F32, tag="mm",
                                               name="psqk")
                                for dd in range(DCH):
                                    nc.tensor.matmul(
                                        ps[:],
                                        lhsT=wqk_sb[:, dd, ec * P:(ec + 1) * P],
                                        rhs=hid_halves[tci][:, dd, :],
                                        start=(dd == 0), stop=(dd == DCH - 1),
                                    )
                                    if dd == DCH // 2:
                                        yield
                                nc.scalar.copy(raw[:, s0:s0 + TCH], ps[:])
                                yield
                            # RoPE: qk = raw*cos + swap32(raw)*sinrot
                            sw = ropep.tile([P, s], BF16, tag="sw", name="sw")
                            nc.sync.dma_start(sw[0:32, :], raw[32:64, :])
                            nc.sync.dma_start(sw[32:64, :], raw[0:32, :])
                            nc.sync.dma_start(sw[64:96, :], raw[96:128, :])
                            nc.sync.dma_start(sw[96:128, :], raw[64:96, :])
                            cp = ropep.tile([P, s], BF16, tag="cp", name="cp")
                            nc.vector.tensor_tensor(cp[:], raw[:], cos_sb[:], MULT)
                            nc.vector.tensor_tensor(sw[:], sw[:], sin_sb[:], MULT)
                            nc.vector.tensor_tensor(qk_t[:, ec, :], cp[:], sw[:], ADD)
                            yield

                    return qk_t, v_t, work()

                def sdpa(bi, qk_t, v_t, filler):
                    """SDPA for batch bi; pops `filler` steps inside kt loops.
                    Returns the normalized attention outputs staging tile."""
                    stage = stagep.tile([hd, h_loc, s], BF16, tag="stage",
                                        name="stage")
                    for pp in range(h_loc // 2):
                        for qt in range(NQT):
                            q0 = qt * QT
                            ps_o = psO.tile([P, 2, QT], F32, tag="pso",
                                            name="pso")
                            for kt in range(KTC):
                                ps_s = psS.tile([P, 2, QT], F32, tag="pss",
                                                name="pss")
                                nc.tensor.matmul(
                                    ps_s[:, 0, :],
                                    lhsT=qk_t[0:64, 2 + pp, kt * P:(kt + 1) * P],
                                    rhs=qk_t[0:64, pp, q0:q0 + QT],
                                    start=True, stop=True,
                                )
                                nc.tensor.matmul(
                                    ps_s[:, 1, :],
                                    lhsT=qk_t[64:128, 2 + pp, kt * P:(kt + 1) * P],
                                    rhs=qk_t[64:128, pp, q0:q0 + QT],
                                    start=True, stop=True, tile_position=(64, 0),
                                )
                                e = expp.tile([P, 2, QT], BF16, tag="exp",
                                              name="e")
                                nc.scalar.activation(e[:, 0, :], ps_s[:, 0, :], EXP)
                                nc.scalar.activation(e[:, 1, :], ps_s[:, 1, :], EXP)
                                for i, hh in enumerate((2 * pp, 2 * pp + 1)):
                                    nc.tensor.matmul(
                                        ps_o[0:hd + 1, i, :],
                                        lhsT=v_t[:, kt,
                                                 hh * (hd + 1):(hh + 1) * (hd + 1)],
                                        rhs=e[:, i, :],
                                        start=(kt == 0), stop=(kt == KTC - 1),
                                    )
                                next(filler, None)
                                next(filler, None)
                            # softmax normalize: ao = ps_o * (1/denominator)
                            dcp = normp.tile([hd + 1, 2, QT], F32, tag="dcp",
                                             name="dcp")
                            nc.scalar.copy(dcp[hd:hd + 1, 0, :],
                                           ps_o[hd:hd + 1, 0, :])
                            nc.scalar.copy(dcp[hd:hd + 1, 1, :],
                                           ps_o[hd:hd + 1, 1, :])
                            dg = normp.tile([2, QT], F32, tag="dg", name="dg")
                            nc.sync.dma_start(dg[:], dcp[hd:hd + 1, :, :])
                            dgr = normp.tile([2, QT], F32, tag="dgr",
                                             name="dgr")
                            nc.vector.reciprocal_approx_fast(dgr[:], dg[:])
                            dgb = normp.tile([2, QT], BF16, tag="dgb",
                                             name="dgb")
                            nc.scalar.copy(dgb[:], dgr[:])
                            rdt = drowp.tile([2, QT], BF16, tag="drow",
                                             name="rdt")
                            nc.sync.dma_start(rdt[:], dgb[:])
                            for i, hh in enumerate((2 * pp, 2 * pp + 1)):
                                rep = normp.tile([hd, QT], BF16, tag="rep",
                                                 name="rep")
                                nc.sync.dma_start(
                                    rep[:], rdt[i:i + 1, :].to_broadcast((hd, QT)))
                                nc.vector.tensor_tensor(
                                    stage[:, hh, q0:q0 + QT],
                                    ps_o[0:hd, i, :], rep[:], MULT)
                    return stage

                asl = aslp.tile([P, ECH, TS], BF16, name="asl")

                # batch pipeline: QKV(0); then SDPA(b) | QKV(b+1) interleaved
                qk_t, v_t, gen = start_qkv(0)
                for _ in gen:
                    pass
                for bi in range(b):
                    if bi + 1 < b:
                        nqk, nv, gen = start_qkv(bi + 1)
                    else:
                        nqk, nv, gen = None, None, iter(())
                    stage = sdpa(bi, qk_t, v_t, gen)
                    for _ in gen:  # drain any remaining QKV work
                        pass
                    qk_t, v_t = nqk, nv
                    # scatter this batch's attention outputs to the A2A input
                    for j in range(n_cores):
                        nc.sync.dma_start(
                            cc_in_h[bi][j]
                                .rearrange("(h p) t -> p h t", p=hd),
                            stage[:, :, j * SH:(j + 1) * SH],
                        )
                    nc.gpsimd.collective_compute(
                        "AllToAll",
                        mybir.AluOpType.bypass,
                        replica_groups=[list(range(n_cores))],
                        ins=[cc_in_h[bi].opt()],
                        outs=[cc_out_h[bi].opt()],
                    )
                    nc.sync.dma_start(
                        asl[:, :, bi * SH:(bi + 1) * SH],
                        cc_out_h[bi][:].rearrange("j (ci p) t -> p (j ci) t",
                                                  p=P),
                    )

                # o_proj on this core's token slice, output [d, tokens]
                for dc in range(NDC):
                    wo_sb = wop.tile([P, ECH, P], BF16, tag="wo", name="wo_sb")
                    nc.sync.dma_start(wo_sb[:], wo_v[:, :, dc * P:(dc + 1) * P])
                    pso = psMM.tile([P, TS], F32, tag="mm", name="pso")
                    for ec in range(ECH):
                        nc.tensor.matmul(
                            pso[:], lhsT=wo_sb[:, ec, :], rhs=asl[:, ec, :],
                            start=(ec == 0), stop=(ec == ECH - 1),
                        )
                    ob = osbp.tile([P, TS], F32, tag="ob", name="ob")
                    nc.scalar.copy(ob[:], pso[:])
                    nc.sync.dma_start(out_sl[dc * P:(dc + 1) * P, :], ob[:])
    nc.finalize()
    return nc


def prep_inputs(cos, sin, hidden_states, w_qkv, w_o,
                b=B, s=S, d=D, h_loc=H_LOC, hd=HD, n_cores=N_CORES):
    """Host-side sharding/layout: returns per-core input maps."""
    bf = ml_dtypes.bfloat16
    cos = np.asarray(cos, dtype=np.float32)
    sin = np.asarray(sin, dtype=np.float32)
    hidden_states = np.asarray(hidden_states, dtype=np.float32)
    w_qkv = np.asarray(w_qkv, dtype=np.float32)
    w_o = np.asarray(w_o, dtype=np.float32)

    T = b * s
    QBLK = h_loc * hd
    HHD = n_cores * QBLK  # total H*HD

    hidden_t = np.ascontiguousarray(hidden_states.reshape(T, d).T).astype(bf)
    w_o_t = np.ascontiguousarray(w_o.T).astype(bf)

    cos_t = cos.T  # [hd, s]
    sin_t = sin.T
    cos2 = np.ascontiguousarray(np.tile(cos_t, (128 // hd, 1))).astype(bf)
    srt = sin_t.copy()
    srt[0:hd // 2] = -sin_t[0:hd // 2]
    sinrot2 = np.ascontiguousarray(np.tile(srt, (128 // hd, 1))).astype(bf)

    maps = []
    for c in range(n_cores):
        wq = w_qkv[c * QBLK:(c + 1) * QBLK] * 0.125
        wk = w_qkv[HHD + c * QBLK:HHD + (c + 1) * QBLK]
        wv = w_qkv[2 * HHD + c * QBLK:2 * HHD + (c + 1) * QBLK]
        w_qk_t = np.ascontiguousarray(np.concatenate([wq, wk], axis=0).T).astype(bf)
        w_v_t = np.ascontiguousarray(wv.T).astype(bf)
        maps.append({
            "hidden_t": hidden_t,
            "w_qk_t": w_qk_t,
            "w_v_t": w_v_t,
            "w_o_t": w_o_t,
            "cos2": cos2,
            "sinrot2": sinrot2,
        })
    return maps


_NC_CACHE = {}


def run(inputs, trace=False, dims=None):
    """Run the distributed kernel. Returns (full_output, BassKernelResults)."""
    dims = dims or dict(b=B, s=S, d=D, h_loc=H_LOC, hd=HD, n_cores=N_CORES)
    key = tuple(sorted(dims.items()))
    if key not in _NC_CACHE:
        _NC_CACHE[key] = build_attention(**dims)
    nc = _NC_CACHE[key]
    maps = prep_inputs(inputs["cos"], inputs["sin"], inputs["hidden_states"],
                       inputs["w_qkv"], inputs["w_o"], **dims)
    res = run_bass_kernel_spmd(nc, maps, list(range(dims["n_cores"])), trace=trace)
    n_cores = dims["n_cores"]
    b, s, d = dims["b"], dims["s"], dims["d"]
    SH = s // n_cores
    out = np.empty((b, s, d), dtype=np.float32)
    for c in range(n_cores):
        sl = res.results[c]["out_sl"]  # [d, b*SH]
        for bi in range(b):
            out[bi, c * SH:(c + 1) * SH, :] = sl[:, bi * SH:(bi + 1) * SH].T
    return out, res


def kernel(**inputs) -> np.ndarray:
    out, _ = run(inputs)
    return out
